# revision 3
# baseline (speedup 1.0000x reference)
"""Bilinear 2x upsample [8,256,256,32] -> [8,512,512,32] fp32 on 8 TRN2 cores.

Sharding: one image per NeuronCore (data-parallel over batch).

Rel-err budget is 2e-2 (global-scale), so the kernel computes in a scaled
fp16 domain and streams part of the output as uint8 via gpsimd cast-DMA:

  device value = k * true_value + 128,   k = 126 / max|img|  (host-picked)

  - N_U8 of the 8 output half-chunks leave via SWDGE (gpsimd) DMAs that
    cast fp16 -> uint8 (round-to-nearest, saturating): 1 byte/elem HBM.
  - The rest leave fp16 on the SP/ACT HWDGE rings: 2 bytes/elem.
  - Host decodes both: (stored - 128) / k.

Per-core pipeline:
  1. H-interp on TensorE: fp16 matmuls, banded row-interp weights
     pre-scaled by 0.25 (exact in fp16), accumulated in fp32 PSUM.
  2. ScalarE (ACT) scaled-copies PSUM segments into 258-slot fp16 SBUF
     buffers (scale = k via a [128,1] runtime AP): S = k * 0.25 * rowinterp.
  3. VectorE W-interp with fast-mode ops (scalar_tensor_tensor has no DVE
     fast modes):
       T      = S * 3 + 128     (tensor_scalar, 4x packed mode)
       out_ev = T + S[k-1]      (tensor_add, 2x packed mode)
       out_od = T + S[k+1]      (tensor_add, 2x packed mode)
  4. Output DMA mix per the N_U8 split.
"""

import numpy as np

import concourse.bass as bass
import concourse.mybir as mybir
import concourse.tile as tile
from concourse import bacc
from concourse.bass_utils import run_bass_kernel_spmd

N_CORES = 8
H = W = 256
OH = OW = 512
C = 32
ROW_FLAT = W * C      # 8192 elems per input row
OUT_FLAT = OW * C     # 16384 elems per output row
SEG = 512             # PSUM bank / matmul free size
SEGS = ROW_FLAT // SEG
FP16 = mybir.dt.float16
U8 = mybir.dt.uint8

N_U8 = 8  # how many of the 8 output half-chunks go out as uint8
BIAS = 128.0
KMAX = 126.0


def _u8_halves(n_u8: int = N_U8) -> set:
    """Interleaved choice of which half-chunk index (0..7) is uint8."""
    return {i for i in range(8)
            if (i * n_u8) // 8 != ((i + 1) * n_u8) // 8}


def _row_interp_matrix() -> np.ndarray:
    scale = np.float32(H / OH)
    rows = np.arange(OH, dtype=np.float32)
    y = (rows + np.float32(0.5)) * scale - np.float32(0.5)
    y = np.maximum(y, np.float32(0.0))
    r0 = np.floor(y).astype(np.int32)
    r1 = r0 + (r0 < W - 1).astype(np.int32)  # reference quirk: guard with W-1
    h0 = (y - r0.astype(np.float32)).astype(np.float32)
    R = np.zeros((OH, H), dtype=np.float32)
    np.add.at(R, (np.arange(OH), r0), np.float32(1.0) - h0)
    np.add.at(R, (np.arange(OH), r1), h0)
    return R


_WPAIRS = [(0, 0), (1, 0), (1, 1), (2, 0), (2, 1), (3, 1)]


def _make_weights() -> np.ndarray:
    R = _row_interp_matrix() * np.float32(0.25)
    mats = []
    for q, t in _WPAIRS:
        blk = R[128 * q:128 * (q + 1), 128 * t:128 * (t + 1)]
        mats.append(np.ascontiguousarray(blk.T))
    return np.concatenate(mats, axis=1).astype(np.float16)


def _build_nc(repeat: int = 1, timing: bool = False,
              n_u8: int = N_U8) -> bass.Bass:
    nc = bacc.Bacc(
        "TRN2",
        target_bir_lowering=False,
        debug=False,
        enable_asserts=False,
        num_devices=N_CORES,
    )
    img_t = nc.dram_tensor("img", [H, ROW_FLAT], FP16, kind="ExternalInput")
    wts = nc.dram_tensor("wts", [128, len(_WPAIRS) * 128], FP16,
                         kind="ExternalInput").ap()
    scl = nc.dram_tensor("scl", [128, 1], mybir.dt.float32,
                         kind="ExternalInput").ap()
    okind = "Internal" if timing else "ExternalOutput"
    out8 = nc.dram_tensor("out8", [OH, OUT_FLAT], U8, kind=okind).ap()
    # out16 only materializes when some half-chunks stay fp16.
    okind16 = okind if n_u8 < 8 else "Internal"
    out16 = nc.dram_tensor("out16", [OH, OUT_FLAT], FP16, kind=okind16).ap()
    probe = None
    if timing:
        probe = nc.dram_tensor("probe", [1, 128], U8,
                               kind="ExternalOutput").ap()

    passes = {0: [0], 1: [1, 2], 2: [3, 4], 3: [5]}
    src_tile = [t for _, t in _WPAIRS]
    u8set = _u8_halves(n_u8)

    with tile.TileContext(nc) as tc:
        with (
            tc.tile_pool(name="wpool", bufs=1) as wpool,
            tc.tile_pool(name="inpool", bufs=1) as inpool,
            tc.tile_pool(name="bpool", bufs=4) as bpool,
            tc.tile_pool(name="tpool", bufs=2) as tpool,
            tc.tile_pool(name="opool", bufs=4) as opool,
            tc.tile_pool(name="pspool", bufs=8, space="PSUM") as pspool,
        ):
            # With all output DMAs on the SWDGE (gpsimd) ring, both HWDGE
            # rings are free for input: SP carries the image, ACT the
            # small weight/scale tensors.
            nw = len(_WPAIRS)
            wall = wpool.tile([128, nw * 128], FP16, tag="wall")
            nc.scalar.dma_start(out=wall[:], in_=wts)
            sclt = wpool.tile([128, 1], mybir.dt.float32, tag="scl")
            nc.scalar.dma_start(out=sclt[:], in_=scl)
            wtiles = [wall[:, 128 * i:128 * (i + 1)] for i in range(nw)]
            # Input loads on the SP HWDGE ring (idle: all output DMAs are
            # SWDGE).  Loaded once, ahead of the body -- same structure the
            # baseline measurement used.
            inall = inpool.tile([128, 2 * ROW_FLAT], FP16, tag="inall")
            for lo, hi, col in ((0, 65, 0), (65, 128, 0), (0, 128, 1)):
                img_src = bass.AP(img_t, (128 * col + lo) * ROW_FLAT,
                                  [[ROW_FLAT, hi - lo], [1, ROW_FLAT]])
                nc.sync.dma_start(
                    out=inall[lo:hi, ROW_FLAT * col:ROW_FLAT * (col + 1)],
                    in_=img_src)
            in_tiles = [inall[:, ROW_FLAT * t:ROW_FLAT * (t + 1)]
                        for t in range(2)]

            def body():
                _emit_body(nc, tc, pspool, bpool, tpool, opool, wtiles,
                           in_tiles, sclt, out8, out16, passes, src_tile,
                           u8set)

            if repeat > 1:
                with tc.For_i(0, repeat, 1, staggered_reset=True):
                    body()
            else:
                body()

            if timing:
                pt = opool.tile([1, 128], U8, tag="probe")
                nc.sync.dma_start(out=pt[:], in_=out8[0:1, 0:128])
                nc.sync.dma_start(out=probe, in_=pt[:])
    nc.compile()
    return nc


def _emit_body(nc, tc, pspool, bpool, tpool, opool, wtiles, in_tiles, sclt,
               out8, out16, passes, src_tile, u8set):
            hw_i = 0  # alternator for HWDGE fp16 output DMAs
            for q in (0, 1, 3, 2):
                bbl = bpool.tile([128, 130 * C], FP16, tag="bbl")
                bbh = bpool.tile([128, 130 * C], FP16, tag="bbh")
                for s in range(SEGS):
                    ps = pspool.tile([128, SEG], mybir.dt.float32, tag="ps")
                    idxs = passes[q]
                    for j, wi in enumerate(idxs):
                        kr = 65 if q == 0 else 128
                        lhsT = wtiles[wi][0:kr, :]
                        rhs = in_tiles[src_tile[wi]][0:kr,
                                                     SEG * s:SEG * (s + 1)]
                        nc.tensor.matmul(
                            ps[:],
                            lhsT,
                            rhs,
                            start=(j == 0),
                            stop=(j == len(idxs) - 1),
                        )
                    # ACT scaled copy = downconvert + quant scale: S = k*B.
                    if s < 8:
                        dst0 = (1 + 16 * s) * C
                        nc.scalar.mul(bbl[:, dst0:dst0 + SEG], ps[:], sclt[:])
                        if s == 0:
                            nc.scalar.copy(out=bbl[:, 0:C], in_=bbl[:, C:2 * C])
                        if s == 7:  # B[127] -> bbh slot 0
                            nc.scalar.mul(bbh[:, 0:C], ps[:, SEG - C:SEG],
                                          sclt[:])
                    else:
                        dst0 = (1 + 16 * (s - 8)) * C
                        nc.scalar.mul(bbh[:, dst0:dst0 + SEG], ps[:], sclt[:])
                        if s == 8:  # B[128] -> bbl slot 129
                            nc.scalar.mul(bbl[:, 129 * C:130 * C],
                                          ps[:, 0:C], sclt[:])
                        if s == SEGS - 1:  # dup B[255] -> bbh slot 129
                            nc.scalar.copy(out=bbh[:, 129 * C:130 * C],
                                           in_=bbh[:, 128 * C:129 * C])
                for h, bbx in ((0, bbl), (1, bbh)):
                    half_i = 2 * q + h  # output half-chunk index 0..7
                    tb = tpool.tile([128, 128 * C], FP16, tag="tb")
                    # T = 3*S + 128 (bias for the uint8 domain; fp16 halves
                    # carry the same bias, host removes it uniformly).
                    nc.vector.tensor_scalar(tb[:], bbx[:, C:C + 128 * C],
                                            3.0, BIAS,
                                            mybir.AluOpType.mult,
                                            mybir.AluOpType.add)
                    if q == 0:
                        # Quarter-granular DMAs: first output earlier.
                        for u in range(2):
                            ot = opool.tile([128, 4096], FP16, tag="ot")
                            o3 = ot[:].rearrange("p (k j) -> p k j", j=2 * C)
                            base = 64 * u * C
                            main = tb[:, base:base + 64 * C].rearrange(
                                "p (k c) -> p k c", c=C)
                            prev = bbx[:, base:base + 64 * C].rearrange(
                                "p (k c) -> p k c", c=C)
                            nxt = bbx[:, base + 2 * C:base + 2 * C + 64 * C
                                      ].rearrange("p (k c) -> p k c", c=C)
                            nc.vector.tensor_add(o3[:, :, 0:C], main, prev)
                            nc.vector.tensor_add(o3[:, :, C:2 * C], main, nxt)
                            u4 = 2 * h + u
                            if half_i in u8set:
                                nc.gpsimd.dma_start(
                                    out=out8[0:128, 4096 * u4:4096 * (u4 + 1)],
                                    in_=ot[:])
                            else:
                                nc.sync.dma_start(
                                    out=out16[0:128,
                                              4096 * u4:4096 * (u4 + 1)],
                                    in_=ot[:])
                        continue
                    ot = opool.tile([128, 8192], FP16, tag="ot")
                    o3 = ot[:].rearrange("p (k j) -> p k j", j=2 * C)
                    main = tb[:].rearrange("p (k c) -> p k c", c=C)
                    prev = bbx[:, 0:128 * C].rearrange("p (k c) -> p k c", c=C)
                    nxt = bbx[:, 2 * C:2 * C + 128 * C].rearrange(
                        "p (k c) -> p k c", c=C)
                    nc.vector.tensor_add(o3[:, :, 0:C], main, prev)
                    nc.vector.tensor_add(o3[:, :, C:2 * C], main, nxt)
                    rows = slice(128 * q, 128 * (q + 1))
                    dst = slice(8192 * h, 8192 * (h + 1))
                    if half_i in u8set:
                        nc.gpsimd.dma_start(out=out8[rows, dst], in_=ot[:])
                    else:
                        dma_eng = nc.sync if hw_i % 2 == 0 else nc.scalar
                        hw_i += 1
                        dma_eng.dma_start(out=out16[rows, dst], in_=ot[:])


_NC_CACHE: dict = {}


def _get_nc() -> bass.Bass:
    if "nc" not in _NC_CACHE:
        _NC_CACHE["nc"] = _build_nc()
    return _NC_CACHE["nc"]


def _run(img: np.ndarray, **kwargs):
    assert img.shape == (N_CORES, H, W, C), img.shape
    wts = _make_weights()
    gmax = float(np.abs(img).max())
    k = np.float32(KMAX / gmax) if gmax > 0 else np.float32(1.0)
    sclv = np.full((128, 1), k, dtype=np.float32)
    img16 = img.astype(np.float16)
    in_maps = [
        {"img": np.ascontiguousarray(img16[i].reshape(H, ROW_FLAT)),
         "wts": wts, "scl": sclv}
        for i in range(N_CORES)
    ]
    res = run_bass_kernel_spmd(_get_nc(), in_maps,
                               core_ids=list(range(N_CORES)), **kwargs)
    u8set = _u8_halves()
    inv = np.float32(1.0) / k
    outs = np.empty((N_CORES, OH, OW, C), dtype=np.float32)
    for i in range(N_CORES):
        o8 = res.results[i]["out8"]
        o16 = res.results[i].get("out16")
        if len(u8set) == 8:
            full = o8.astype(np.float32)
        else:
            full = np.empty((OH, OUT_FLAT), dtype=np.float32)
            for half_i in range(8):
                q, hh = divmod(half_i, 2)
                cols = slice(8192 * hh, 8192 * (hh + 1))
                rows = slice(128 * q, 128 * (q + 1))
                src = o8 if half_i in u8set else o16
                full[rows, cols] = src[rows, cols].astype(np.float32)
        outs[i] = ((full - np.float32(BIAS)) * inv).reshape(OH, OW, C)
    return outs, res


def kernel(**inputs) -> np.ndarray:
    img = np.ascontiguousarray(np.asarray(inputs["img"], dtype=np.float32))
    outs, _ = _run(img)
    return outs
